# revision 7
# baseline (speedup 1.0000x reference)
"""Trainium2 Bass kernel for ConditionalNeuralNetwork (MoE-style routed MLP).

Strategy (expert-parallel over combos, data-parallel within a combo):
  - Host computes combo idx = 2*flags[:,0] + flags[:,1] per row, groups rows
    by combo, and splits each combo's rows across 2 of the 8 cores.
  - Each core receives only ITS head's weights, so the device kernel is a
    plain dense MLP 256 -> 1024 -> 1024 -> 512 -> 256 -> 1 with relu/sigmoid.
    This halves head FLOPs vs computing all 4 heads densely.
  - Matmul inputs are bf16 (full PE rate, FWL weight loads); accumulation
    and bias+activation epilogues are fp32 in PSUM/ACT.
  - Host scatters per-core outputs back to original row order.
"""

import os
import sys

import ml_dtypes
import numpy as np

for _p in ("/opt/trn_rl_repo", "/root/.axon_site/_ro/trn_rl_repo"):
    if os.path.isdir(_p) and _p not in sys.path:
        sys.path.append(_p)

import concourse.bacc as bacc
import concourse.bass as bass
import concourse.tile as tile
from concourse import mybir
from concourse.bass import MemorySpace
from concourse.bass_utils import run_bass_kernel_spmd

F32 = mybir.dt.float32
BF16 = mybir.dt.bfloat16
AF = mybir.ActivationFunctionType
NPBF16 = ml_dtypes.bfloat16

B, D_IN = 16384, 256
S1, S2 = 1024, 1024
H1, H2 = 512, 256
C = 4
NCORES = 8
CAP = 2176  # rows per core (max needed with seed-0 counts: 2080)
CHUNK = 512

_nc_cache = None
_last_results = None


def _build():
    """Build the single-core MLP program (SPMD across 8 cores)."""
    nc = bacc.Bacc("TRN2", target_bir_lowering=False, debug=False)

    def din(name, shape, dt=BF16):
        return nc.dram_tensor(name, list(shape), dt, kind="ExternalInput").ap()

    xT = din("xT", [128, 2, CAP])          # x rows, feature-major tiled
    w1 = din("w1", [128, 2, S1])
    b1 = din("b1", [128, S1 // 128], F32)
    w2 = din("w2", [128, 8, S2])
    b2 = din("b2", [128, S2 // 128], F32)
    hw1 = din("hw1", [128, 8, H1])
    hb1 = din("hb1", [128, H1 // 128], F32)
    hw2 = din("hw2", [128, 4, H2])
    hb2 = din("hb2", [128, H2 // 128], F32)
    hw3 = din("hw3", [128, 2])
    hb3 = din("hb3", [1, 1], F32)
    out = nc.dram_tensor("out", [1, CAP], F32, kind="ExternalOutput").ap()

    n_chunks = (CAP + CHUNK - 1) // CHUNK
    chunks = [(i * CHUNK, min(CHUNK, CAP - i * CHUNK)) for i in range(n_chunks)]

    with tile.TileContext(nc) as tc:
        with tc.tile_pool(name="weights", bufs=1) as wp, \
             tc.tile_pool(name="acts", bufs=2) as ap, \
             tc.tile_pool(name="outs", bufs=2) as op, \
             tc.tile_pool(name="psum", bufs=4, space=MemorySpace.PSUM) as pp, \
             tc.tile_pool(name="psum_l", bufs=2, space=MemorySpace.PSUM) as plp:

            w1s = wp.tile([128, 2, S1], BF16, tag="w1s")
            w2s = wp.tile([128, 8, S2], BF16, tag="w2s")
            hw1s = wp.tile([128, 8, H1], BF16, tag="hw1s")
            hw2s = wp.tile([128, 4, H2], BF16, tag="hw2s")
            hw3s = wp.tile([128, 2], BF16, tag="hw3s")
            b1s = wp.tile([128, S1 // 128], F32, tag="b1s")
            b2s = wp.tile([128, S2 // 128], F32, tag="b2s")
            hb1s = wp.tile([128, H1 // 128], F32, tag="hb1s")
            hb2s = wp.tile([128, H2 // 128], F32, tag="hb2s")
            hb3s = wp.tile([1, 1], F32, tag="hb3s")

            for sb, dr in ((w1s, w1), (b1s, b1), (w2s, w2), (b2s, b2),
                           (hw1s, hw1), (hb1s, hb1), (hw2s, hw2),
                           (hb2s, hb2), (hw3s, hw3), (hb3s, hb3)):
                nc.sync.dma_start(out=sb[:], in_=dr[:])

            for n0, N in chunks:
                xt = ap.tile([128, 2, CHUNK], BF16, tag="xt")
                nc.sync.dma_start(out=xt[:, :, :N], in_=xT[:, :, n0:n0 + N])

                # L1: 256 -> 1024, relu
                h1 = ap.tile([128, 8, CHUNK], BF16, tag="h1")
                for m in range(8):
                    ps = pp.tile([128, CHUNK], F32, tag="ps")
                    for k in range(2):
                        nc.tensor.matmul(
                            ps[:, :N], w1s[:, k, m * 128:(m + 1) * 128],
                            xt[:, k, :N], start=(k == 0), stop=(k == 1))
                    nc.scalar.activation(h1[:, m, :N], ps[:, :N], AF.Relu,
                                         bias=b1s[:, m:m + 1])

                # L2: 1024 -> 1024, relu
                h2 = ap.tile([128, 8, CHUNK], BF16, tag="h2")
                for m in range(8):
                    ps = pp.tile([128, CHUNK], F32, tag="ps")
                    for k in range(8):
                        nc.tensor.matmul(
                            ps[:, :N], w2s[:, k, m * 128:(m + 1) * 128],
                            h1[:, k, :N], start=(k == 0), stop=(k == 7))
                    nc.scalar.activation(h2[:, m, :N], ps[:, :N], AF.Relu,
                                         bias=b2s[:, m:m + 1])

                # Head L1: 1024 -> 512, relu
                a1 = ap.tile([128, 4, CHUNK], BF16, tag="a1")
                for m in range(4):
                    ps = pp.tile([128, CHUNK], F32, tag="ps")
                    for k in range(8):
                        nc.tensor.matmul(
                            ps[:, :N], hw1s[:, k, m * 128:(m + 1) * 128],
                            h2[:, k, :N], start=(k == 0), stop=(k == 7))
                    nc.scalar.activation(a1[:, m, :N], ps[:, :N], AF.Relu,
                                         bias=hb1s[:, m:m + 1])

                # Head L2: 512 -> 256, relu
                a2 = ap.tile([128, 2, CHUNK], BF16, tag="a2")
                for m in range(2):
                    ps = pp.tile([128, CHUNK], F32, tag="ps")
                    for k in range(4):
                        nc.tensor.matmul(
                            ps[:, :N], hw2s[:, k, m * 128:(m + 1) * 128],
                            a1[:, k, :N], start=(k == 0), stop=(k == 3))
                    nc.scalar.activation(a2[:, m, :N], ps[:, :N], AF.Relu,
                                         bias=hb2s[:, m:m + 1])

                # Head L3: 256 -> 1, sigmoid
                psl = plp.tile([1, CHUNK], F32, tag="psl")
                for k in range(2):
                    nc.tensor.matmul(psl[:, :N], hw3s[:, k:k + 1],
                                     a2[:, k, :N],
                                     start=(k == 0), stop=(k == 1))
                ot = op.tile([1, CHUNK], F32, tag="ot")
                nc.scalar.activation(ot[:, :N], psl[:, :N], AF.Sigmoid,
                                     bias=hb3s[:1, :1])
                nc.sync.dma_start(out=out[:, n0:n0 + N], in_=ot[:, :N])

    nc.compile()
    return nc


def _get_nc():
    global _nc_cache
    if _nc_cache is None:
        _nc_cache = _build()
    return _nc_cache


def _tile_k(w, ktiles):
    """[K, M] -> [128, ktiles, M] bf16 with K = ktiles*128, K idx = k*128+p."""
    k, m = w.shape
    assert k == ktiles * 128
    return np.ascontiguousarray(
        w.reshape(ktiles, 128, m).transpose(1, 0, 2).astype(NPBF16))


def _tile_b(b):
    """[M] -> [128, M/128] f32; column m holds bias for m-tile m."""
    m = b.shape[0]
    return np.ascontiguousarray(b.reshape(m // 128, 128).T.astype(np.float32))


def kernel(**inputs):
    global _last_results
    x = np.asarray(inputs["x"], dtype=np.float32)
    ff = np.asarray(inputs["feature_flags"]).astype(np.int64)
    idx = ff[:, 0] * 2 + ff[:, 1]

    W1 = np.asarray(inputs["W1"], np.float32)
    b1 = np.asarray(inputs["b1"], np.float32)
    W2 = np.asarray(inputs["W2"], np.float32)
    b2 = np.asarray(inputs["b2"], np.float32)
    HW1 = np.asarray(inputs["HW1"], np.float32)
    Hb1 = np.asarray(inputs["Hb1"], np.float32)
    HW2 = np.asarray(inputs["HW2"], np.float32)
    Hb2 = np.asarray(inputs["Hb2"], np.float32)
    HW3 = np.asarray(inputs["HW3"], np.float32)
    Hb3 = np.asarray(inputs["Hb3"], np.float32)

    # Row assignment: combo c -> cores 2c, 2c+1.
    row_sets = []
    for c in range(C):
        rows = np.nonzero(idx == c)[0]
        h = (len(rows) + 1) // 2
        row_sets.append(rows[:h])
        row_sets.append(rows[h:])
    for r in row_sets:
        if len(r) > CAP:
            raise ValueError(f"combo shard of {len(r)} rows exceeds CAP={CAP}")

    w1t = _tile_k(W1, 2)
    w2t = _tile_k(W2, 8)
    b1t = _tile_b(b1)
    b2t = _tile_b(b2)
    hw1t = [_tile_k(HW1[c], 8) for c in range(C)]
    hw2t = [_tile_k(HW2[c], 4) for c in range(C)]
    hw3t = [np.ascontiguousarray(
        HW3[c][:, 0].reshape(2, 128).T.astype(NPBF16)) for c in range(C)]
    hb1t = [_tile_b(Hb1[c]) for c in range(C)]
    hb2t = [_tile_b(Hb2[c]) for c in range(C)]
    hb3t = [np.asarray(Hb3[c]).reshape(1, 1).astype(np.float32)
            for c in range(C)]

    in_maps = []
    for d, rows in enumerate(row_sets):
        c = d // 2
        n = len(rows)
        xt = np.zeros((128, 2, CAP), NPBF16)
        if n:
            xt[:, :, :n] = x[rows].T.reshape(2, 128, n).transpose(
                1, 0, 2).astype(NPBF16)
        in_maps.append({
            "xT": xt,
            "w1": w1t, "b1": b1t, "w2": w2t, "b2": b2t,
            "hw1": hw1t[c], "hb1": hb1t[c],
            "hw2": hw2t[c], "hb2": hb2t[c],
            "hw3": hw3t[c], "hb3": hb3t[c],
        })

    nc = _get_nc()
    res = run_bass_kernel_spmd(nc, in_maps, core_ids=list(range(NCORES)))
    _last_results = res

    out = np.empty(B, np.float32)
    for d, rows in enumerate(row_sets):
        if len(rows):
            out[rows] = res.results[d]["out"][0, :len(rows)]
    return out


# revision 8
# speedup vs baseline: 1.0917x; 1.0917x over previous
"""Trainium2 Bass kernel for ConditionalNeuralNetwork (MoE-style routed MLP).

Strategy (expert-parallel over combos, data-parallel within a combo):
  - Host computes combo idx = 2*flags[:,0] + flags[:,1] per row, groups rows
    by combo, and splits each combo's rows across 2 of the 8 cores.
  - Each core receives only ITS head's weights, so the device kernel is a
    plain dense MLP 256 -> 1024 -> 1024 -> 512 -> 256 -> 1 with relu/sigmoid.
    This halves head FLOPs vs computing all 4 heads densely.
  - Matmul inputs are bf16 (full PE rate, FWL weight loads); accumulation
    and bias+activation epilogues are fp32 in PSUM/ACT.
  - Host scatters per-core outputs back to original row order.
"""

import os
import sys

import ml_dtypes
import numpy as np

for _p in ("/opt/trn_rl_repo", "/root/.axon_site/_ro/trn_rl_repo"):
    if os.path.isdir(_p) and _p not in sys.path:
        sys.path.append(_p)

import concourse.bacc as bacc
import concourse.bass as bass
import concourse.tile as tile
from concourse import mybir
from concourse.bass import MemorySpace
from concourse.bass_utils import run_bass_kernel_spmd

F32 = mybir.dt.float32
BF16 = mybir.dt.bfloat16
AF = mybir.ActivationFunctionType
NPBF16 = ml_dtypes.bfloat16

B, D_IN = 16384, 256
S1, S2 = 1024, 1024
H1, H2 = 512, 256
C = 4
NCORES = 8
CAP = 2176  # rows per core (max needed with seed-0 counts: 2080)
CHUNK = 512

_nc_cache = None
_last_results = None


def _build():
    """Build the single-core MLP program (SPMD across 8 cores)."""
    nc = bacc.Bacc("TRN2", target_bir_lowering=False, debug=False)

    def din(name, shape, dt=BF16):
        return nc.dram_tensor(name, list(shape), dt, kind="ExternalInput").ap()

    xT = din("xT", [128, 2, CAP])          # x rows, feature-major tiled
    w1 = din("w1", [128, 2, S1])
    b1 = din("b1", [128, S1 // 128], F32)
    w2 = din("w2", [128, 8, S2])
    b2 = din("b2", [128, S2 // 128], F32)
    hw1 = din("hw1", [128, 8, H1])
    hb1 = din("hb1", [128, H1 // 128], F32)
    hw2 = din("hw2", [128, 4, H2])
    hb2 = din("hb2", [128, H2 // 128], F32)
    hw3 = din("hw3", [128, 2])
    hb3 = din("hb3", [1, 1], F32)
    out = nc.dram_tensor("out", [1, CAP], F32, kind="ExternalOutput").ap()

    n_chunks = (CAP + CHUNK - 1) // CHUNK
    chunks = [(i * CHUNK, min(CHUNK, CAP - i * CHUNK)) for i in range(n_chunks)]
    ALU = mybir.AluOpType

    with tile.TileContext(nc) as tc:
        with tc.tile_pool(name="weights", bufs=1) as wp, \
             tc.tile_pool(name="xin", bufs=n_chunks) as xp, \
             tc.tile_pool(name="acts", bufs=3) as ap, \
             tc.tile_pool(name="outs", bufs=2) as op, \
             tc.tile_pool(name="psum", bufs=6, space=MemorySpace.PSUM) as pp, \
             tc.tile_pool(name="psum_l", bufs=2, space=MemorySpace.PSUM) as plp:

            w1s = wp.tile([128, 2, S1], BF16, tag="w1s")
            w2s = wp.tile([128, 8, S2], BF16, tag="w2s")
            hw1s = wp.tile([128, 8, H1], BF16, tag="hw1s")
            hw2s = wp.tile([128, 4, H2], BF16, tag="hw2s")
            hw3s = wp.tile([128, 2], BF16, tag="hw3s")
            b1s = wp.tile([128, S1 // 128], F32, tag="b1s")
            b2s = wp.tile([128, S2 // 128], F32, tag="b2s")
            hb1s = wp.tile([128, H1 // 128], F32, tag="hb1s")
            hb2s = wp.tile([128, H2 // 128], F32, tag="hb2s")
            hb3s = wp.tile([1, 1], F32, tag="hb3s")

            # DMA order = consumption order: small consts + L1 weights +
            # all x chunks first, then per-k slices of the big matrices so
            # compute can start as soon as its slice lands.
            for sb, dr in ((w1s, w1), (b1s, b1), (b2s, b2), (hb1s, hb1),
                           (hb2s, hb2), (hb3s, hb3), (hw3s, hw3)):
                nc.sync.dma_start(out=sb[:], in_=dr[:])
            xts = []
            for n0, N in chunks:
                xt = xp.tile([128, 2, CHUNK], BF16, tag="xt")
                nc.sync.dma_start(out=xt[:, :, :N], in_=xT[:, :, n0:n0 + N])
                xts.append(xt)
            for k in range(8):
                nc.sync.dma_start(out=w2s[:, k, :], in_=w2[:, k, :])
            for k in range(8):
                nc.sync.dma_start(out=hw1s[:, k, :], in_=hw1[:, k, :])
            nc.sync.dma_start(out=hw2s[:], in_=hw2[:])

            # Bias+relu epilogue, alternating between ACT and DVE so neither
            # engine gates the PE during low-arithmetic layers.
            epi_n = [0]

            def epilogue(dst, src, bias_ap):
                if epi_n[0] % 2 == 0:
                    nc.scalar.activation(dst, src, AF.Relu, bias=bias_ap)
                else:
                    nc.vector.tensor_scalar(
                        dst, src, bias_ap, 0.0, ALU.add, ALU.max)
                epi_n[0] += 1

            for ci, (n0, N) in enumerate(chunks):
                xt = xts[ci]

                # L1: 256 -> 1024, relu
                h1 = ap.tile([128, 8, CHUNK], BF16, tag="h1")
                for m in range(8):
                    ps = pp.tile([128, CHUNK], F32, tag="ps")
                    for k in range(2):
                        nc.tensor.matmul(
                            ps[:, :N], w1s[:, k, m * 128:(m + 1) * 128],
                            xt[:, k, :N], start=(k == 0), stop=(k == 1))
                    epilogue(h1[:, m, :N], ps[:, :N], b1s[:, m:m + 1])

                # L2: 1024 -> 1024, relu
                h2 = ap.tile([128, 8, CHUNK], BF16, tag="h2")
                for m in range(8):
                    ps = pp.tile([128, CHUNK], F32, tag="ps")
                    for k in range(8):
                        nc.tensor.matmul(
                            ps[:, :N], w2s[:, k, m * 128:(m + 1) * 128],
                            h1[:, k, :N], start=(k == 0), stop=(k == 7))
                    epilogue(h2[:, m, :N], ps[:, :N], b2s[:, m:m + 1])

                # Head L1: 1024 -> 512, relu
                a1 = ap.tile([128, 4, CHUNK], BF16, tag="a1")
                for m in range(4):
                    ps = pp.tile([128, CHUNK], F32, tag="ps")
                    for k in range(8):
                        nc.tensor.matmul(
                            ps[:, :N], hw1s[:, k, m * 128:(m + 1) * 128],
                            h2[:, k, :N], start=(k == 0), stop=(k == 7))
                    epilogue(a1[:, m, :N], ps[:, :N], hb1s[:, m:m + 1])

                # Head L2: 512 -> 256, relu
                a2 = ap.tile([128, 2, CHUNK], BF16, tag="a2")
                for m in range(2):
                    ps = pp.tile([128, CHUNK], F32, tag="ps")
                    for k in range(4):
                        nc.tensor.matmul(
                            ps[:, :N], hw2s[:, k, m * 128:(m + 1) * 128],
                            a1[:, k, :N], start=(k == 0), stop=(k == 3))
                    epilogue(a2[:, m, :N], ps[:, :N], hb2s[:, m:m + 1])

                # Head L3: 256 -> 1, sigmoid
                psl = plp.tile([1, CHUNK], F32, tag="psl")
                for k in range(2):
                    nc.tensor.matmul(psl[:, :N], hw3s[:, k:k + 1],
                                     a2[:, k, :N],
                                     start=(k == 0), stop=(k == 1))
                ot = op.tile([1, CHUNK], F32, tag="ot")
                nc.scalar.activation(ot[:, :N], psl[:, :N], AF.Sigmoid,
                                     bias=hb3s[:1, :1])
                nc.sync.dma_start(out=out[:, n0:n0 + N], in_=ot[:, :N])

    nc.compile()
    return nc


def _get_nc():
    global _nc_cache
    if _nc_cache is None:
        _nc_cache = _build()
    return _nc_cache


def _tile_k(w, ktiles):
    """[K, M] -> [128, ktiles, M] bf16 with K = ktiles*128, K idx = k*128+p."""
    k, m = w.shape
    assert k == ktiles * 128
    return np.ascontiguousarray(
        w.reshape(ktiles, 128, m).transpose(1, 0, 2).astype(NPBF16))


def _tile_b(b):
    """[M] -> [128, M/128] f32; column m holds bias for m-tile m."""
    m = b.shape[0]
    return np.ascontiguousarray(b.reshape(m // 128, 128).T.astype(np.float32))


def kernel(**inputs):
    global _last_results
    x = np.asarray(inputs["x"], dtype=np.float32)
    ff = np.asarray(inputs["feature_flags"]).astype(np.int64)
    idx = ff[:, 0] * 2 + ff[:, 1]

    W1 = np.asarray(inputs["W1"], np.float32)
    b1 = np.asarray(inputs["b1"], np.float32)
    W2 = np.asarray(inputs["W2"], np.float32)
    b2 = np.asarray(inputs["b2"], np.float32)
    HW1 = np.asarray(inputs["HW1"], np.float32)
    Hb1 = np.asarray(inputs["Hb1"], np.float32)
    HW2 = np.asarray(inputs["HW2"], np.float32)
    Hb2 = np.asarray(inputs["Hb2"], np.float32)
    HW3 = np.asarray(inputs["HW3"], np.float32)
    Hb3 = np.asarray(inputs["Hb3"], np.float32)

    # Row assignment: combo c -> cores 2c, 2c+1.
    row_sets = []
    for c in range(C):
        rows = np.nonzero(idx == c)[0]
        h = (len(rows) + 1) // 2
        row_sets.append(rows[:h])
        row_sets.append(rows[h:])
    for r in row_sets:
        if len(r) > CAP:
            raise ValueError(f"combo shard of {len(r)} rows exceeds CAP={CAP}")

    w1t = _tile_k(W1, 2)
    w2t = _tile_k(W2, 8)
    b1t = _tile_b(b1)
    b2t = _tile_b(b2)
    hw1t = [_tile_k(HW1[c], 8) for c in range(C)]
    hw2t = [_tile_k(HW2[c], 4) for c in range(C)]
    hw3t = [np.ascontiguousarray(
        HW3[c][:, 0].reshape(2, 128).T.astype(NPBF16)) for c in range(C)]
    hb1t = [_tile_b(Hb1[c]) for c in range(C)]
    hb2t = [_tile_b(Hb2[c]) for c in range(C)]
    hb3t = [np.asarray(Hb3[c]).reshape(1, 1).astype(np.float32)
            for c in range(C)]

    in_maps = []
    for d, rows in enumerate(row_sets):
        c = d // 2
        n = len(rows)
        xt = np.zeros((128, 2, CAP), NPBF16)
        if n:
            xt[:, :, :n] = x[rows].T.reshape(2, 128, n).transpose(
                1, 0, 2).astype(NPBF16)
        in_maps.append({
            "xT": xt,
            "w1": w1t, "b1": b1t, "w2": w2t, "b2": b2t,
            "hw1": hw1t[c], "hb1": hb1t[c],
            "hw2": hw2t[c], "hb2": hb2t[c],
            "hw3": hw3t[c], "hb3": hb3t[c],
        })

    nc = _get_nc()
    res = run_bass_kernel_spmd(nc, in_maps, core_ids=list(range(NCORES)))
    _last_results = res

    out = np.empty(B, np.float32)
    for d, rows in enumerate(row_sets):
        if len(rows):
            out[rows] = res.results[d]["out"][0, :len(rows)]
    return out


# revision 19
# speedup vs baseline: 1.1578x; 1.0605x over previous
"""Trainium2 Bass kernel for ConditionalNeuralNetwork (MoE-style routed MLP).

Strategy (expert-parallel over combos, data-parallel within a combo):
  - Host computes combo idx = 2*flags[:,0] + flags[:,1] per row, groups rows
    by combo, and splits each combo's rows across 2 of the 8 cores.
  - Each core receives only ITS head's weights, so the device kernel is a
    plain dense MLP 256 -> 1024 -> 1024 -> 512 -> 256 -> 1 with relu/sigmoid.
    This halves head FLOPs vs computing all 4 heads densely.
  - Matmul inputs are bf16 (full PE rate, FWL weight loads); accumulation
    and bias+activation epilogues are fp32 in PSUM/ACT.
  - Host scatters per-core outputs back to original row order.
"""

import os
import sys

import ml_dtypes
import numpy as np

for _p in ("/opt/trn_rl_repo", "/root/.axon_site/_ro/trn_rl_repo"):
    if os.path.isdir(_p) and _p not in sys.path:
        sys.path.append(_p)

import concourse.bacc as bacc
import concourse.bass as bass
import concourse.tile as tile
from concourse import mybir
from concourse.bass import MemorySpace
from concourse.bass_utils import run_bass_kernel_spmd

F32 = mybir.dt.float32
BF16 = mybir.dt.bfloat16
AF = mybir.ActivationFunctionType
NPBF16 = ml_dtypes.bfloat16

B, D_IN = 16384, 256
S1, S2 = 1024, 1024
H1, H2 = 512, 256
C = 4
NCORES = 8
N_CHUNKS = 5
CAP = 2080  # rows per core (max needed with seed-0 counts: 2080)
CHUNK = CAP // N_CHUNKS  # 416: keeps every matmul MM-bound (not LDW-bound)
WARMUP_MMS = 30  # dependency-free PE warm-up matmuls at kernel start

_nc_cache = {}
_last_results = None


def _build(cap=CAP):
    """Build the single-core MLP program (SPMD across 8 cores)."""
    nc = bacc.Bacc("TRN2", target_bir_lowering=False, debug=False)

    def din(name, shape, dt=BF16):
        return nc.dram_tensor(name, list(shape), dt, kind="ExternalInput").ap()

    xT = din("xT", [128, 2, cap])          # x rows, feature-major tiled
    w1 = din("w1", [128, 2, S1])
    w2 = din("w2", [128, 8, S2])
    hw1 = din("hw1", [128, 8, H1])
    hw2 = din("hw2", [128, 4, H2])
    hw3 = din("hw3", [128, 2])
    # biases packed into one tensor: [b1(8) | b2(8) | hb1(4) | hb2(2) | hb3]
    cst = din("consts", [128, 23], F32)
    out = nc.dram_tensor("out", [1, cap], F32, kind="ExternalOutput").ap()

    n_chunks = N_CHUNKS
    chunk = cap // n_chunks
    assert chunk * n_chunks == cap and chunk % 32 == 0
    chunks = [(i * chunk, chunk) for i in range(n_chunks)]
    ALU = mybir.AluOpType

    with tile.TileContext(nc) as tc:
        with tc.tile_pool(name="weights", bufs=1) as wp, \
             tc.tile_pool(name="xin", bufs=n_chunks) as xp, \
             tc.tile_pool(name="acts", bufs=3) as ap, \
             tc.tile_pool(name="outs", bufs=2) as op, \
             tc.tile_pool(name="psum", bufs=6, space=MemorySpace.PSUM) as pp, \
             tc.tile_pool(name="psum_l", bufs=2, space=MemorySpace.PSUM) as plp:

            w1s = wp.tile([128, 2, S1], BF16, tag="w1s")
            w2s = wp.tile([128, 8, S2], BF16, tag="w2s")
            hw1s = wp.tile([128, 8, H1], BF16, tag="hw1s")
            hw2s = wp.tile([128, 4, H2], BF16, tag="hw2s")
            hw3s = wp.tile([128, 2], BF16, tag="hw3s")
            csts = wp.tile([128, 23], F32, tag="csts")
            b1s = csts[:, 0:8]
            b2s = csts[:, 8:16]
            hb1s = csts[:, 16:20]
            hb2s = csts[:, 20:22]
            hb3s = csts[:1, 22:23]

            # Two DMA rings in parallel: weights stream on SP (sync),
            # x chunks + consts on ACT (scalar), each in consumption order.
            for k in range(2):
                nc.sync.dma_start(out=w1s[:, k, :], in_=w1[:, k, :])
            xts = []
            for n0, N in chunks:
                xt = xp.tile([128, 2, chunk], BF16, tag="xt")
                nc.scalar.dma_start(out=xt[:, :, :N], in_=xT[:, :, n0:n0 + N])
                xts.append(xt)
                if len(xts) == 1:
                    nc.scalar.dma_start(out=csts[:], in_=cst[:])
            for k in range(8):
                nc.sync.dma_start(out=w2s[:, k, :], in_=w2[:, k, :])
            for k in range(8):
                nc.sync.dma_start(out=hw1s[:, k, :], in_=hw1[:, k, :])
            nc.sync.dma_start(out=hw2s[:], in_=hw2[:])
            nc.sync.dma_start(out=hw3s[:], in_=hw3[:])

            # PE warm-up: dependency-free matmuls fill the initial DMA-wait
            # window and release the HAM clock throttle (~3.4us of busy PE
            # needed for 1.2 -> 2.4 GHz) before the real matmuls arrive.
            if WARMUP_MMS:
                wut = wp.tile([128, 128], BF16, tag="wut")
                nc.vector.memset(wut[:], 0.0)
                wups = plp.tile([1, chunk], F32, tag="psl")
                for _ in range(WARMUP_MMS):
                    nc.tensor.matmul(wups[:1, :128], wut[:, 0:1],
                                     wut[:, :128], start=True, stop=True)

            # Bias+relu epilogue, alternating between ACT and DVE so neither
            # engine gates the PE during low-arithmetic layers.
            epi_n = [0]

            def epilogue(dst, src, bias_ap):
                if epi_n[0] % 2 == 0:
                    nc.scalar.activation(dst, src, AF.Relu, bias=bias_ap)
                else:
                    nc.vector.tensor_scalar(
                        dst, src, bias_ap, 0.0, ALU.add, ALU.max)
                epi_n[0] += 1

            for ci, (n0, N) in enumerate(chunks):
                xt = xts[ci]

                # L1: 256 -> 1024, relu
                h1 = ap.tile([128, 8, chunk], BF16, tag="h1")
                for m in range(8):
                    ps = pp.tile([128, chunk], F32, tag="ps")
                    for k in range(2):
                        nc.tensor.matmul(
                            ps[:, :N], w1s[:, k, m * 128:(m + 1) * 128],
                            xt[:, k, :N], start=(k == 0), stop=(k == 1))
                    epilogue(h1[:, m, :N], ps[:, :N], b1s[:, m:m + 1])

                # L2: 1024 -> 1024, relu
                h2 = ap.tile([128, 8, chunk], BF16, tag="h2")
                for m in range(8):
                    ps = pp.tile([128, chunk], F32, tag="ps")
                    for k in range(8):
                        nc.tensor.matmul(
                            ps[:, :N], w2s[:, k, m * 128:(m + 1) * 128],
                            h1[:, k, :N], start=(k == 0), stop=(k == 7))
                    epilogue(h2[:, m, :N], ps[:, :N], b2s[:, m:m + 1])

                # Head L1: 1024 -> 512, relu
                a1 = ap.tile([128, 4, chunk], BF16, tag="a1")
                for m in range(4):
                    ps = pp.tile([128, chunk], F32, tag="ps")
                    for k in range(8):
                        nc.tensor.matmul(
                            ps[:, :N], hw1s[:, k, m * 128:(m + 1) * 128],
                            h2[:, k, :N], start=(k == 0), stop=(k == 7))
                    epilogue(a1[:, m, :N], ps[:, :N], hb1s[:, m:m + 1])

                # Head L2: 512 -> 256, relu
                a2 = ap.tile([128, 2, chunk], BF16, tag="a2")
                for m in range(2):
                    ps = pp.tile([128, chunk], F32, tag="ps")
                    for k in range(4):
                        nc.tensor.matmul(
                            ps[:, :N], hw2s[:, k, m * 128:(m + 1) * 128],
                            a1[:, k, :N], start=(k == 0), stop=(k == 3))
                    epilogue(a2[:, m, :N], ps[:, :N], hb2s[:, m:m + 1])

                # Head L3: 256 -> 1, sigmoid
                psl = plp.tile([1, chunk], F32, tag="psl")
                for k in range(2):
                    nc.tensor.matmul(psl[:, :N], hw3s[:, k:k + 1],
                                     a2[:, k, :N],
                                     start=(k == 0), stop=(k == 1))
                ot = op.tile([1, chunk], F32, tag="ot")
                nc.scalar.activation(ot[:, :N], psl[:, :N], AF.Sigmoid,
                                     bias=hb3s[:1, :1])
                nc.sync.dma_start(out=out[:, n0:n0 + N], in_=ot[:, :N])

    nc.compile()
    return nc


def _get_nc(cap=CAP):
    if cap not in _nc_cache:
        _nc_cache[cap] = _build(cap)
    return _nc_cache[cap]


def _tile_k(w, ktiles):
    """[K, M] -> [128, ktiles, M] bf16 with K = ktiles*128, K idx = k*128+p."""
    k, m = w.shape
    assert k == ktiles * 128
    return np.ascontiguousarray(
        w.reshape(ktiles, 128, m).transpose(1, 0, 2).astype(NPBF16))


def _tile_b(b):
    """[M] -> [128, M/128] f32; column m holds bias for m-tile m."""
    m = b.shape[0]
    return np.ascontiguousarray(b.reshape(m // 128, 128).T.astype(np.float32))


def _make_in_maps(inputs):
    x = np.asarray(inputs["x"], dtype=np.float32)
    ff = np.asarray(inputs["feature_flags"]).astype(np.int64)
    idx = ff[:, 0] * 2 + ff[:, 1]

    W1 = np.asarray(inputs["W1"], np.float32)
    b1 = np.asarray(inputs["b1"], np.float32)
    W2 = np.asarray(inputs["W2"], np.float32)
    b2 = np.asarray(inputs["b2"], np.float32)
    HW1 = np.asarray(inputs["HW1"], np.float32)
    Hb1 = np.asarray(inputs["Hb1"], np.float32)
    HW2 = np.asarray(inputs["HW2"], np.float32)
    Hb2 = np.asarray(inputs["Hb2"], np.float32)
    HW3 = np.asarray(inputs["HW3"], np.float32)
    Hb3 = np.asarray(inputs["Hb3"], np.float32)

    # Row assignment: combo c -> cores 2c, 2c+1.
    row_sets = []
    for c in range(C):
        rows = np.nonzero(idx == c)[0]
        h = (len(rows) + 1) // 2
        row_sets.append(rows[:h])
        row_sets.append(rows[h:])
    max_shard = max(len(r) for r in row_sets)
    # cap = smallest multiple of 32*N_CHUNKS that fits every shard
    step = 32 * N_CHUNKS
    cap = max(CAP, -(-max_shard // step) * step)

    w1t = _tile_k(W1, 2)
    w2t = _tile_k(W2, 8)
    hw1t = [_tile_k(HW1[c], 8) for c in range(C)]
    hw2t = [_tile_k(HW2[c], 4) for c in range(C)]
    hw3t = [np.ascontiguousarray(
        HW3[c][:, 0].reshape(2, 128).T.astype(NPBF16)) for c in range(C)]
    cstt = []
    for c in range(C):
        cst = np.zeros((128, 23), np.float32)
        cst[:, 0:8] = _tile_b(b1)
        cst[:, 8:16] = _tile_b(b2)
        cst[:, 16:20] = _tile_b(Hb1[c])
        cst[:, 20:22] = _tile_b(Hb2[c])
        cst[:, 22] = np.float32(Hb3[c][0])
        cstt.append(cst)

    in_maps = []
    for d, rows in enumerate(row_sets):
        c = d // 2
        n = len(rows)
        xt = np.zeros((128, 2, cap), NPBF16)
        if n:
            xt[:, :, :n] = x[rows].T.reshape(2, 128, n).transpose(
                1, 0, 2).astype(NPBF16)
        in_maps.append({
            "xT": xt,
            "w1": w1t, "w2": w2t,
            "hw1": hw1t[c], "hw2": hw2t[c], "hw3": hw3t[c],
            "consts": cstt[c],
        })

    return in_maps, row_sets, cap


def kernel(**inputs):
    global _last_results
    in_maps, row_sets, cap = _make_in_maps(inputs)
    nc = _get_nc(cap)
    res = run_bass_kernel_spmd(nc, in_maps, core_ids=list(range(NCORES)))
    _last_results = res

    out = np.empty(B, np.float32)
    for d, rows in enumerate(row_sets):
        if len(rows):
            out[rows] = res.results[d]["out"][0, :len(rows)]
    return out
